# revision 31
# baseline (speedup 1.0000x reference)
"""Causal self-attention (B=2, T=2048, C=1024, H=16) on 8 trn2 NeuronCores.

Sharding (Megatron-style):
  - tensor-parallel over heads: core p owns heads {2p, 2p+1}.  Each core
    computes Q^T/K^T/V^T for its 2 heads from the full x, then causal
    attention (streaming softmax without max-subtraction; the denominator
    comes from a ones-column appended to V).
  - per (batch, 1024-token part): an AllToAll redistributes attention
    outputs so core p holds all 1024 channels for tokens [256p, 256p+256)
    of each batch; collectives fire as soon as their two q-chunks finish.
  - projection: each core computes the full output projection for its two
    256-token slices and writes a disjoint [512, 1024] output block.

Schedule notes (iteration 2):
  - S^T matmuls are emitted as 64-partition pairs (heads at partitions
    0-63/64-127) which run concurrently on disjoint PE row groups.
  - S tiles live in PSUM as bf16 (1 bank) with bufs=3 so three k-tiles
    are in flight; the exp (ACT) latency is hidden behind the PE stream.
  - causal diag-cut: S/exp only computed on the causally-needed q range.
  - softmax epilogue is fully on-chip: DVE reciprocal of the denominator
    row, PE K=1-matmul broadcast of it across 64 partitions, fused
    normalize (PSUM o_t x recip -> bf16 anorm).
  - the last AllToAll's window is filled with the three earlier
    projection parts; proj(1,1) is the only post-collective PE work.
  - x / W_qkv / W_proj are pre-arranged on host so every DMA line is
    contiguous per partition.
"""

import numpy as np

B, T, C, H, D = 2, 2048, 1024, 16, 64
NCORES = 8
HL = H // NCORES        # heads per core = 2
TOK = B * T             # 4096 global tokens
TSL = TOK // NCORES     # 512 output tokens per core (256 per batch)
SL = 256                # per-batch token slice per core
P = 128
CT = C // P             # 8 contraction tiles
NQC = T // 512          # 4 q-chunks per batch
NKT = T // P            # 16 k-tiles per batch
SCALE = D ** -0.5

_CACHE = {}


def _build_nc():
    import concourse.bass as bass
    import concourse.mybir as mybir
    from concourse import bacc
    from concourse.tile import TileContext

    f32 = mybir.dt.float32
    bf16 = mybir.dt.bfloat16
    AF = mybir.ActivationFunctionType
    ALU = mybir.AluOpType

    nc = bacc.Bacc(
        "TRN2", target_bir_lowering=False, debug=False, num_devices=NCORES
    )

    xH = nc.dram_tensor("xH", [P, B * NQC * CT * 512], bf16, kind="ExternalInput")
    wq = nc.dram_tensor("wq", [P, CT * 3 * P], bf16, kind="ExternalInput")
    bqkv = nc.dram_tensor("bqkv", [3 * P], f32, kind="ExternalInput")
    wp = nc.dram_tensor("wp", [P, CT * C], bf16, kind="ExternalInput")
    bp = nc.dram_tensor("bp", [C], f32, kind="ExternalInput")
    tri = nc.dram_tensor("tri", [P, P], bf16, kind="ExternalInput")
    ident = nc.dram_tensor("ident", [P, P], bf16, kind="ExternalInput")
    onesb = nc.dram_tensor("onesb", [P, 64], bf16, kind="ExternalInput")
    onesf = nc.dram_tensor("onesf", [P, 64], f32, kind="ExternalInput")
    y = nc.dram_tensor("y", [TSL, C], f32, kind="ExternalOutput")

    with TileContext(nc, num_cores=NCORES) as tc:
        from contextlib import ExitStack

        with ExitStack() as ctx:
            const = ctx.enter_context(tc.tile_pool(name="const", bufs=1))
            persist = ctx.enter_context(tc.tile_pool(name="persist", bufs=1))
            dram = ctx.enter_context(tc.tile_pool(name="dram", bufs=1, space="DRAM"))

            # ---- constants (small ones on gpsimd queue; weights on sync)
            tri_sb = const.tile([P, P], bf16)
            id_sb = const.tile([P, P], bf16)
            bq_sb = const.tile([P, 3], f32)
            ones_sb = const.tile([P, 64], bf16)
            ones1_sb = const.tile([P, 64], f32)
            bpb_sb = const.tile([P, C], f32)
            w_sb = const.tile([P, CT, 3 * P], bf16)     # W_qkv^T tiles
            wp_sb = const.tile([P, CT, C], bf16)        # W_proj^T (loaded late)
            nc.gpsimd.dma_start(tri_sb[:], tri[:])
            nc.gpsimd.dma_start(id_sb[:], ident[:])
            nc.gpsimd.dma_start(bq_sb[:], bqkv.rearrange("(et p) -> p et", p=P))
            nc.gpsimd.dma_start(ones_sb[:], onesb[:])
            nc.gpsimd.dma_start(ones1_sb[:], onesf[:])
            # W_qkv^T halves on two idle queues (x chunk 0 takes sync+gpsimd)
            wv = w_sb.rearrange("p ct e -> p (ct e)")
            nc.scalar.dma_start(wv[:, 0:4 * 3 * P], wq[:, 0:4 * 3 * P])
            nc.scalar.dma_start(wv[:, 4 * 3 * P:], wq[:, 4 * 3 * P:])

            # ---- persistent activations (per batch)
            qTb = [persist.tile([P, T], bf16, name=f"qT{b}") for b in range(B)]
            kTb = [persist.tile([P, T], bf16, name=f"kT{b}") for b in range(B)]
            vTb = [persist.tile([P, T], bf16, name=f"vT{b}") for b in range(B)]
            # V with ones column, per batch: [128 tok, k-tile, 2*65]
            vaugb = [persist.tile([P, NKT, 2 * 65], bf16, name=f"vaug{b}")
                     for b in range(B)]
            # normalized A^T per local head (each head at partitions 0-63)
            anorm = [persist.tile([64, TOK], bf16, name=f"anorm{h}")
                     for h in range(HL)]
            rbounce = dram.tile([B * NQC * HL, 512], f32, name="rbounce")

            # ones columns of vaug written once per batch
            for b in range(B):
                vo = vaugb[b].rearrange("p k (h e) -> p (k h) e", e=65)
                nc.gpsimd.tensor_copy(vo[:, :, 64:65],
                                      ones_sb[:, 0:2 * NKT]
                                      .rearrange("p (k o) -> p k o", o=1))

            pools = [
                tc.tile_pool(name="sps", bufs=2, space="PSUM"),
                tc.tile_pool(name="ops", bufs=2, space="PSUM"),
                tc.tile_pool(name="mm", bufs=2, space="PSUM"),
                tc.tile_pool(name="pT", bufs=3),
                tc.tile_pool(name="xslab", bufs=3),
                tc.tile_pool(name="rp", bufs=2),
                tc.tile_pool(name="afull", bufs=2),
                tc.tile_pool(name="ysb", bufs=2),
            ]
            sps, ops, mm, ppool, xpool, rpool, apool, ypool = (
                ctx.enter_context(p) for p in pools)

            def qkv_chunk(b, c4, split=False):
                """qkv^T for one 512-token chunk of batch b + V transposes."""
                c = b * NQC + c4
                xsl = xpool.tile([P, CT, 512], bf16, tag="x")
                xv = xsl.rearrange("p ct t -> p (ct t)")
                s0 = c * CT * 512
                if split:
                    half = CT * 512 // 2
                    nc.sync.dma_start(xv[:, 0:half], xH[:, s0:s0 + half])
                    nc.gpsimd.dma_start(
                        xv[:, half:], xH[:, s0 + half:s0 + CT * 512])
                elif c4 == 1:
                    nc.scalar.dma_start(xv[:], xH[:, s0:s0 + CT * 512])
                else:
                    nc.sync.dma_start(xv[:], xH[:, s0:s0 + CT * 512])
                for et, dstl in enumerate((qTb, kTb, vTb)):
                    ps = mm.tile([P, 512], mybir.dt.float32, tag="mm")
                    for ct in range(CT):
                        nc.tensor.matmul(
                            ps[:],
                            lhsT=w_sb[:, ct, et * P:(et + 1) * P],
                            rhs=xsl[:, ct, :],
                            start=(ct == 0),
                            stop=(ct == CT - 1),
                        )
                    nc.vector.tensor_scalar_add(
                        dstl[b][:, c4 * 512:(c4 + 1) * 512],
                        ps[:],
                        bq_sb[:, et:et + 1],
                    )
                # V^T -> V for this chunk's 4 k-tiles (PE transpose) into vaug
                for kt in range(c4 * 4, c4 * 4 + 4):
                    tp = mm.tile([P, P], bf16, tag="mm")
                    nc.tensor.transpose(
                        tp[:],
                        vTb[b][:, kt * P:(kt + 1) * P],
                        id_sb[:],
                    )
                    nc.vector.tensor_copy(
                        vaugb[b][:, kt, 0:2 * 65]
                        .rearrange("p (h e) -> p h e", h=2)[:, :, 0:64],
                        tp.rearrange("p (h e) -> p h e", h=2),
                    )

            def attention_qc(b, qc):
                q0 = qc * 512
                nk = 4 * qc + 4                   # causal k-tiles
                ot = [ops.tile([65, 512], mybir.dt.float32, tag="o",
                               name=f"ot{h}")
                      for h in range(HL)]

                def emit_pv(ki, pt, lo):
                    for h in range(HL):
                        nc.tensor.matmul(
                            ot[h][:, lo:512],
                            lhsT=vaugb[b][:, ki, h * 65:h * 65 + 65],
                            rhs=pt[:, h, lo:512],
                            start=(ki == 0),
                            stop=(ki == nk - 1),
                        )

                pend = None
                for ki in range(nk):
                    off = ki * P - q0
                    lo = max(0, off)
                    sp = sps.tile([P, HL, 512], mybir.dt.float32, tag="s")
                    for h in range(HL):
                        hp = slice(64 * h, 64 * h + 64)
                        nc.tensor.matmul(
                            sp[:, h, lo:512],
                            lhsT=kTb[b][hp, ki * P:(ki + 1) * P],
                            rhs=qTb[b][hp, q0 + lo:q0 + 512],
                            start=True,
                            stop=True,
                        )
                    pt = ppool.tile([P, HL, 512], bf16, tag="p")
                    nc.scalar.activation(
                        pt[:, :, lo:512], sp[:, :, lo:512], AF.Exp, scale=SCALE,
                    )
                    if off >= 0:
                        for h in range(HL):
                            nc.vector.tensor_tensor(
                                pt[:, h, off:off + P],
                                pt[:, h, off:off + P],
                                tri_sb[:],
                                ALU.mult,
                            )
                    if pend is not None:
                        emit_pv(*pend)
                    pend = (ki, pt, lo)
                emit_pv(*pend)

                # epilogue: normalize this q-chunk into anorm.
                # (only >=64-partition DVE ops; at most one PSUM operand;
                #  recip row broadcast across partitions via SBUF->SBUF DMA)
                for h in range(HL):
                    c0 = b * T + q0
                    dn = rpool.tile([65, 512], mybir.dt.float32, tag="dn")
                    nc.vector.tensor_copy(dn[:], ot[h][:])
                    dnr = rpool.tile([65, 512], mybir.dt.float32, tag="dnr")
                    nc.vector.reciprocal_approx_fast(dnr[:], dn[:])
                    rr = (b * NQC + qc) * HL + h
                    nc.sync.dma_start(
                        rbounce[rr:rr + 1, :], dnr[64:65, :])
                    rb = rpool.tile([64, 512], mybir.dt.float32, tag="rb")
                    nc.sync.dma_start(
                        rb[:], rbounce[rr:rr + 1, :].to_broadcast((64, 512)))
                    nc.vector.tensor_tensor(
                        anorm[h][:, c0:c0 + 512],
                        dn[0:64, :],
                        rb[:],
                        ALU.mult,
                    )

            def a2a_part(b, part):
                a2a_in = dram.tile([NCORES * P, P], bf16,
                                   name=f"a2a_in{b}_{part}")
                a2a_out = dram.tile([NCORES * P, P], bf16,
                                    name=f"a2a_out{b}_{part}")
                a2a_v = a2a_in.rearrange("(j ee) t -> ee j t", j=NCORES)
                for h in range(HL):
                    c0 = b * T + 1024 * part
                    nc.sync.dma_start(
                        a2a_v[64 * h:64 * h + 64],
                        anorm[h][:, c0:c0 + 1024]
                        .rearrange("e (j t) -> e j t", j=NCORES),
                    )
                nc.gpsimd.collective_compute(
                    "AllToAll",
                    ALU.bypass,
                    replica_groups=[list(range(NCORES))],
                    ins=[a2a_in.opt()],
                    outs=[a2a_out.opt()],
                )
                return a2a_out

            def proj_part(b, part, a2a_out):
                afull = apool.tile([P, NCORES, P], bf16, tag="af")
                nc.sync.dma_start(
                    afull[:],
                    a2a_out.rearrange("(i e) t -> e i t", i=NCORES),
                )
                ysb = ypool.tile([P, C], mybir.dt.float32, tag="ysb")
                for fc in range(C // 512):
                    ps = mm.tile([P, 512], mybir.dt.float32, tag="mm")
                    for i in range(NCORES):
                        nc.tensor.matmul(
                            ps[:],
                            lhsT=afull[:, i, :],
                            rhs=wp_sb[:, i, fc * 512:(fc + 1) * 512],
                            start=(i == 0),
                            stop=(i == NCORES - 1),
                        )
                    nc.vector.tensor_tensor(
                        ysb[:, fc * 512:(fc + 1) * 512],
                        ps[:],
                        bpb_sb[:, fc * 512:(fc + 1) * 512],
                        ALU.add,
                    )
                r0 = b * SL + part * P
                nc.scalar.dma_start(y[r0:r0 + P, :], ysb[:])

            # ---- schedule
            qkv_chunk(0, 0, split=True)
            qkv_chunk(0, 1)
            attention_qc(0, 0)
            qkv_chunk(0, 2)
            attention_qc(0, 1)
            out00 = a2a_part(0, 0)
            qkv_chunk(0, 3)
            attention_qc(0, 2)
            qkv_chunk(1, 0)
            attention_qc(0, 3)
            out01 = a2a_part(0, 1)
            qkv_chunk(1, 1)
            nc.sync.dma_start(
                wp_sb.rearrange("p ct f -> p (ct f)")[:], wp[:]
            )
            nc.scalar.dma_start(
                bpb_sb[:], bp.rearrange("(o c) -> o c", o=1).to_broadcast((P, C))
            )
            attention_qc(1, 0)
            qkv_chunk(1, 2)
            attention_qc(1, 1)
            out10 = a2a_part(1, 0)
            qkv_chunk(1, 3)
            attention_qc(1, 2)
            proj_part(0, 0, out00)
            proj_part(0, 1, out01)
            attention_qc(1, 3)
            proj_part(1, 0, out10)
            out11 = a2a_part(1, 1)
            proj_part(1, 1, out11)
    nc.compile()
    return nc


def _prep_inputs(x, W_qkv, b_qkv, W_proj, b_proj):
    x = np.asarray(x, dtype=np.float32)
    W_qkv = np.asarray(W_qkv, dtype=np.float32)
    b_qkv = np.asarray(b_qkv, dtype=np.float32)
    W_proj = np.asarray(W_proj, dtype=np.float32)
    b_proj = np.asarray(b_proj, dtype=np.float32)

    import ml_dtypes
    bf = ml_dtypes.bfloat16
    # x -> [p, chunk, ct, t] contiguous per partition
    xt = x.reshape(B * NQC, 512, CT, P)
    xHn = np.ascontiguousarray(
        xt.transpose(3, 0, 2, 1).reshape(P, B * NQC * CT * 512)).astype(bf)
    wpH = np.ascontiguousarray(
        W_proj.T.reshape(CT, P, C).transpose(1, 0, 2).reshape(P, CT * C)
    ).astype(bf)
    tri = np.triu(np.ones((P, P), dtype=np.float32)).astype(bf)
    ident = np.eye(P, dtype=np.float32).astype(bf)
    onesb = np.ones((P, 64), dtype=np.float32).astype(bf)
    onesf = np.ones((P, 64), dtype=np.float32)

    in_maps = []
    for p in range(NCORES):
        rows = np.r_[128 * p:128 * p + 128,
                     C + 128 * p:C + 128 * p + 128,
                     2 * C + 128 * p:2 * C + 128 * p + 128]
        wslice = W_qkv[rows]                      # [384, 1024]
        bslice = np.ascontiguousarray(b_qkv[rows])
        wqH = np.ascontiguousarray(
            wslice.T.reshape(CT, P, 3 * P).transpose(1, 0, 2)
            .reshape(P, CT * 3 * P)).astype(bf)
        in_maps.append({
            "xH": xHn,
            "wq": wqH,
            "bqkv": bslice,
            "wp": wpH,
            "bp": b_proj,
            "tri": tri,
            "ident": ident,
            "onesb": onesb,
            "onesf": onesf,
        })
    return in_maps


def kernel(x, W_qkv, b_qkv, W_proj, b_proj, _trace=False):
    from concourse import bass_utils

    if "nc" not in _CACHE:
        _CACHE["nc"] = _build_nc()
    nc = _CACHE["nc"]
    in_maps = _prep_inputs(x, W_qkv, b_qkv, W_proj, b_proj)
    res = bass_utils.run_bass_kernel_spmd(
        nc, in_maps, core_ids=list(range(NCORES)), trace=_trace,
    )
    _CACHE["last_result"] = res
    # core p rows: [b*256 + part*128 + i] = batch b, token
    # b*2048 + part*1024 + 128*p + i
    yfull = np.empty((B, T, C), dtype=np.float32)
    for p, rmap in enumerate(res.results):
        yp = rmap["y"]
        for b in range(B):
            for part in range(2):
                g0 = part * 1024 + 128 * p
                r0 = b * SL + part * P
                yfull[b, g0:g0 + P] = yp[r0:r0 + P]
    return yfull


# revision 32
# speedup vs baseline: 1.0024x; 1.0024x over previous
"""Causal self-attention (B=2, T=2048, C=1024, H=16) on 8 trn2 NeuronCores.

Sharding (Megatron-style):
  - tensor-parallel over heads: core p owns heads {2p, 2p+1}.  Each core
    computes Q^T/K^T/V^T for its 2 heads from the full x, then causal
    attention (streaming softmax without max-subtraction; the denominator
    comes from a ones-column appended to V).
  - per (batch, 1024-token part): an AllToAll redistributes attention
    outputs so core p holds all 1024 channels for tokens [256p, 256p+256)
    of each batch; collectives fire as soon as their two q-chunks finish.
  - projection: each core computes the full output projection for its two
    256-token slices and writes a disjoint [512, 1024] output block.

Schedule notes (iteration 2):
  - S^T matmuls are emitted as 64-partition pairs (heads at partitions
    0-63/64-127) which run concurrently on disjoint PE row groups.
  - S tiles live in PSUM as bf16 (1 bank) with bufs=3 so three k-tiles
    are in flight; the exp (ACT) latency is hidden behind the PE stream.
  - causal diag-cut: S/exp only computed on the causally-needed q range.
  - softmax epilogue is fully on-chip: DVE reciprocal of the denominator
    row, PE K=1-matmul broadcast of it across 64 partitions, fused
    normalize (PSUM o_t x recip -> bf16 anorm).
  - the last AllToAll's window is filled with the three earlier
    projection parts; proj(1,1) is the only post-collective PE work.
  - x / W_qkv / W_proj are pre-arranged on host so every DMA line is
    contiguous per partition.
"""

import numpy as np

B, T, C, H, D = 2, 2048, 1024, 16, 64
NCORES = 8
HL = H // NCORES        # heads per core = 2
TOK = B * T             # 4096 global tokens
TSL = TOK // NCORES     # 512 output tokens per core (256 per batch)
SL = 256                # per-batch token slice per core
P = 128
CT = C // P             # 8 contraction tiles
NQC = T // 512          # 4 q-chunks per batch
NKT = T // P            # 16 k-tiles per batch
SCALE = D ** -0.5

_CACHE = {}


def _build_nc():
    import concourse.bass as bass
    import concourse.mybir as mybir
    from concourse import bacc
    from concourse.tile import TileContext

    f32 = mybir.dt.float32
    bf16 = mybir.dt.bfloat16
    AF = mybir.ActivationFunctionType
    ALU = mybir.AluOpType

    nc = bacc.Bacc(
        "TRN2", target_bir_lowering=False, debug=False, num_devices=NCORES
    )

    xH = nc.dram_tensor("xH", [P, B * NQC * CT * 512], bf16, kind="ExternalInput")
    wq = nc.dram_tensor("wq", [P, CT * 3 * P], bf16, kind="ExternalInput")
    bqkv = nc.dram_tensor("bqkv", [3 * P], f32, kind="ExternalInput")
    wp = nc.dram_tensor("wp", [P, CT * C], bf16, kind="ExternalInput")
    bp = nc.dram_tensor("bp", [C], f32, kind="ExternalInput")
    tri = nc.dram_tensor("tri", [P, P], bf16, kind="ExternalInput")
    ident = nc.dram_tensor("ident", [P, P], bf16, kind="ExternalInput")
    onesb = nc.dram_tensor("onesb", [P, 64], bf16, kind="ExternalInput")
    onesf = nc.dram_tensor("onesf", [P, 64], f32, kind="ExternalInput")
    y = nc.dram_tensor("y", [TSL, C], f32, kind="ExternalOutput")

    with TileContext(nc, num_cores=NCORES) as tc:
        from contextlib import ExitStack

        with ExitStack() as ctx:
            const = ctx.enter_context(tc.tile_pool(name="const", bufs=1))
            persist = ctx.enter_context(tc.tile_pool(name="persist", bufs=1))
            dram = ctx.enter_context(tc.tile_pool(name="dram", bufs=1, space="DRAM"))

            # ---- constants (small ones on gpsimd queue; weights on sync)
            tri_sb = const.tile([P, P], bf16)
            id_sb = const.tile([P, P], bf16)
            bq_sb = const.tile([P, 3], f32)
            ones_sb = const.tile([P, 64], bf16)
            ones1_sb = const.tile([P, 64], f32)
            bpb_sb = const.tile([P, C], f32)
            w_sb = const.tile([P, CT, 3 * P], bf16)     # W_qkv^T tiles
            wp_sb = const.tile([P, CT, C], bf16)        # W_proj^T (loaded late)
            nc.gpsimd.dma_start(tri_sb[:], tri[:])
            nc.gpsimd.dma_start(id_sb[:], ident[:])
            nc.gpsimd.dma_start(bq_sb[:], bqkv.rearrange("(et p) -> p et", p=P))
            nc.gpsimd.dma_start(ones_sb[:], onesb[:])
            nc.gpsimd.dma_start(ones1_sb[:], onesf[:])
            # W_qkv^T halves on two idle queues (x chunk 0 takes sync+gpsimd)
            wv = w_sb.rearrange("p ct e -> p (ct e)")
            nc.scalar.dma_start(wv[:, 0:4 * 3 * P], wq[:, 0:4 * 3 * P])
            nc.scalar.dma_start(wv[:, 4 * 3 * P:], wq[:, 4 * 3 * P:])

            # ---- persistent activations (per batch)
            qTb = [persist.tile([P, T], bf16, name=f"qT{b}") for b in range(B)]
            kTb = [persist.tile([P, T], bf16, name=f"kT{b}") for b in range(B)]
            vTb = [persist.tile([P, T], bf16, name=f"vT{b}") for b in range(B)]
            # V with ones column, per batch: [128 tok, k-tile, 2*65]
            vaugb = [persist.tile([P, NKT, 2 * 65], bf16, name=f"vaug{b}")
                     for b in range(B)]
            # normalized A^T per local head (each head at partitions 0-63)
            anorm = [persist.tile([64, TOK], bf16, name=f"anorm{h}")
                     for h in range(HL)]
            rbounce = dram.tile([B * NQC * HL, 512], f32, name="rbounce")

            # ones columns of vaug written once per batch
            for b in range(B):
                vo = vaugb[b].rearrange("p k (h e) -> p (k h) e", e=65)
                nc.gpsimd.tensor_copy(vo[:, :, 64:65],
                                      ones_sb[:, 0:2 * NKT]
                                      .rearrange("p (k o) -> p k o", o=1))

            pools = [
                tc.tile_pool(name="sps", bufs=2, space="PSUM"),
                tc.tile_pool(name="ops", bufs=2, space="PSUM"),
                tc.tile_pool(name="mm", bufs=2, space="PSUM"),
                tc.tile_pool(name="pT", bufs=3),
                tc.tile_pool(name="xslab", bufs=3),
                tc.tile_pool(name="rp", bufs=2),
                tc.tile_pool(name="afull", bufs=2),
                tc.tile_pool(name="ysb", bufs=2),
            ]
            sps, ops, mm, ppool, xpool, rpool, apool, ypool = (
                ctx.enter_context(p) for p in pools)

            def qkv_chunk(b, c4, split=False):
                """qkv^T for one 512-token chunk of batch b + V transposes."""
                c = b * NQC + c4
                xsl = xpool.tile([P, CT, 512], bf16, tag="x")
                xv = xsl.rearrange("p ct t -> p (ct t)")
                s0 = c * CT * 512
                if split:
                    half = CT * 512 // 2
                    nc.sync.dma_start(xv[:, 0:half], xH[:, s0:s0 + half])
                    nc.gpsimd.dma_start(
                        xv[:, half:], xH[:, s0 + half:s0 + CT * 512])
                elif c4 == 1:
                    nc.scalar.dma_start(xv[:], xH[:, s0:s0 + CT * 512])
                else:
                    nc.sync.dma_start(xv[:], xH[:, s0:s0 + CT * 512])
                for et, dstl in enumerate((qTb, kTb, vTb)):
                    ps = mm.tile([P, 512], mybir.dt.float32, tag="mm")
                    for ct in range(CT):
                        nc.tensor.matmul(
                            ps[:],
                            lhsT=w_sb[:, ct, et * P:(et + 1) * P],
                            rhs=xsl[:, ct, :],
                            start=(ct == 0),
                            stop=(ct == CT - 1),
                        )
                    nc.vector.tensor_scalar_add(
                        dstl[b][:, c4 * 512:(c4 + 1) * 512],
                        ps[:],
                        bq_sb[:, et:et + 1],
                    )
                # V^T -> V for this chunk's 4 k-tiles (PE transpose) into vaug
                for kt in range(c4 * 4, c4 * 4 + 4):
                    tp = mm.tile([P, P], bf16, tag="mm")
                    nc.tensor.transpose(
                        tp[:],
                        vTb[b][:, kt * P:(kt + 1) * P],
                        id_sb[:],
                    )
                    nc.vector.tensor_copy(
                        vaugb[b][:, kt, 0:2 * 65]
                        .rearrange("p (h e) -> p h e", h=2)[:, :, 0:64],
                        tp.rearrange("p (h e) -> p h e", h=2),
                    )

            def attention_qc(b, qc):
                q0 = qc * 512
                nk = 4 * qc + 4                   # causal k-tiles
                ot = [ops.tile([65, 512], mybir.dt.float32, tag="o",
                               name=f"ot{h}")
                      for h in range(HL)]

                def emit_pv(ki, pt, lo):
                    for h in range(HL):
                        nc.tensor.matmul(
                            ot[h][:, lo:512],
                            lhsT=vaugb[b][:, ki, h * 65:h * 65 + 65],
                            rhs=pt[:, h, lo:512],
                            start=(ki == 0),
                            stop=(ki == nk - 1),
                        )

                pend = None
                for ki in range(nk):
                    off = ki * P - q0
                    lo = max(0, off)
                    sp = sps.tile([P, HL, 512], mybir.dt.float32, tag="s")
                    for h in range(HL):
                        hp = slice(64 * h, 64 * h + 64)
                        nc.tensor.matmul(
                            sp[:, h, lo:512],
                            lhsT=kTb[b][hp, ki * P:(ki + 1) * P],
                            rhs=qTb[b][hp, q0 + lo:q0 + 512],
                            start=True,
                            stop=True,
                        )
                    pt = ppool.tile([P, HL, 512], bf16, tag="p")
                    nc.scalar.activation(
                        pt[:, :, lo:512], sp[:, :, lo:512], AF.Exp, scale=SCALE,
                    )
                    if off >= 0:
                        for h in range(HL):
                            nc.vector.tensor_tensor(
                                pt[:, h, off:off + P],
                                pt[:, h, off:off + P],
                                tri_sb[:],
                                ALU.mult,
                            )
                    if pend is not None:
                        emit_pv(*pend)
                    pend = (ki, pt, lo)
                emit_pv(*pend)

                # epilogue: normalize this q-chunk into anorm.
                # (only >=64-partition DVE ops; at most one PSUM operand;
                #  recip row broadcast across partitions via SBUF->SBUF DMA)
                for h in range(HL):
                    c0 = b * T + q0
                    dn = rpool.tile([65, 512], mybir.dt.float32, tag="dn")
                    nc.vector.tensor_copy(dn[:], ot[h][:])
                    dnr = rpool.tile([65, 512], mybir.dt.float32, tag="dnr")
                    nc.vector.reciprocal_approx_fast(dnr[:], dn[:])
                    rr = (b * NQC + qc) * HL + h
                    nc.sync.dma_start(
                        rbounce[rr:rr + 1, :], dnr[64:65, :])
                    rb = rpool.tile([64, 512], mybir.dt.float32, tag="rb")
                    nc.sync.dma_start(
                        rb[:], rbounce[rr:rr + 1, :].to_broadcast((64, 512)))
                    nc.vector.tensor_tensor(
                        anorm[h][:, c0:c0 + 512],
                        dn[0:64, :],
                        rb[:],
                        ALU.mult,
                    )

            def a2a_part(b, part):
                a2a_in = dram.tile([NCORES * P, P], bf16,
                                   name=f"a2a_in{b}_{part}")
                a2a_out = dram.tile([NCORES * P, P], bf16,
                                    name=f"a2a_out{b}_{part}")
                a2a_v = a2a_in.rearrange("(j ee) t -> ee j t", j=NCORES)
                for h in range(HL):
                    c0 = b * T + 1024 * part
                    nc.sync.dma_start(
                        a2a_v[64 * h:64 * h + 64],
                        anorm[h][:, c0:c0 + 1024]
                        .rearrange("e (j t) -> e j t", j=NCORES),
                    )
                nc.gpsimd.collective_compute(
                    "AllToAll",
                    ALU.bypass,
                    replica_groups=[list(range(NCORES))],
                    ins=[a2a_in.opt()],
                    outs=[a2a_out.opt()],
                )
                return a2a_out

            def proj_part(b, part, a2a_out):
                afull = apool.tile([P, NCORES, P], bf16, tag="af")
                nc.sync.dma_start(
                    afull[:],
                    a2a_out.rearrange("(i e) t -> e i t", i=NCORES),
                )
                ysb = ypool.tile([P, C], mybir.dt.float32, tag="ysb")
                for fc in range(C // 512):
                    ps = mm.tile([P, 512], mybir.dt.float32, tag="mm")
                    for i in range(NCORES):
                        nc.tensor.matmul(
                            ps[:],
                            lhsT=afull[:, i, :],
                            rhs=wp_sb[:, i, fc * 512:(fc + 1) * 512],
                            start=(i == 0),
                            stop=(i == NCORES - 1),
                        )
                    nc.vector.tensor_tensor(
                        ysb[:, fc * 512:(fc + 1) * 512],
                        ps[:],
                        bpb_sb[:, fc * 512:(fc + 1) * 512],
                        ALU.add,
                    )
                r0 = b * SL + part * P
                nc.scalar.dma_start(y[r0:r0 + P, :], ysb[:])

            # ---- schedule
            qkv_chunk(0, 0, split=True)
            qkv_chunk(0, 1)
            attention_qc(0, 0)
            qkv_chunk(0, 2)
            attention_qc(0, 1)
            out00 = a2a_part(0, 0)
            qkv_chunk(0, 3)
            attention_qc(0, 2)
            qkv_chunk(1, 0)
            attention_qc(0, 3)
            out01 = a2a_part(0, 1)
            qkv_chunk(1, 1)
            nc.sync.dma_start(
                wp_sb.rearrange("p ct f -> p (ct f)")[:], wp[:]
            )
            nc.scalar.dma_start(
                bpb_sb[:], bp.rearrange("(o c) -> o c", o=1).to_broadcast((P, C))
            )
            attention_qc(1, 0)
            qkv_chunk(1, 2)
            attention_qc(1, 1)
            out10 = a2a_part(1, 0)
            qkv_chunk(1, 3)
            attention_qc(1, 2)
            proj_part(0, 0, out00)
            attention_qc(1, 3)
            proj_part(0, 1, out01)
            proj_part(1, 0, out10)
            out11 = a2a_part(1, 1)
            proj_part(1, 1, out11)
    nc.compile()
    return nc


def _prep_inputs(x, W_qkv, b_qkv, W_proj, b_proj):
    x = np.asarray(x, dtype=np.float32)
    W_qkv = np.asarray(W_qkv, dtype=np.float32)
    b_qkv = np.asarray(b_qkv, dtype=np.float32)
    W_proj = np.asarray(W_proj, dtype=np.float32)
    b_proj = np.asarray(b_proj, dtype=np.float32)

    import ml_dtypes
    bf = ml_dtypes.bfloat16
    # x -> [p, chunk, ct, t] contiguous per partition
    xt = x.reshape(B * NQC, 512, CT, P)
    xHn = np.ascontiguousarray(
        xt.transpose(3, 0, 2, 1).reshape(P, B * NQC * CT * 512)).astype(bf)
    wpH = np.ascontiguousarray(
        W_proj.T.reshape(CT, P, C).transpose(1, 0, 2).reshape(P, CT * C)
    ).astype(bf)
    tri = np.triu(np.ones((P, P), dtype=np.float32)).astype(bf)
    ident = np.eye(P, dtype=np.float32).astype(bf)
    onesb = np.ones((P, 64), dtype=np.float32).astype(bf)
    onesf = np.ones((P, 64), dtype=np.float32)

    in_maps = []
    for p in range(NCORES):
        rows = np.r_[128 * p:128 * p + 128,
                     C + 128 * p:C + 128 * p + 128,
                     2 * C + 128 * p:2 * C + 128 * p + 128]
        wslice = W_qkv[rows]                      # [384, 1024]
        bslice = np.ascontiguousarray(b_qkv[rows])
        wqH = np.ascontiguousarray(
            wslice.T.reshape(CT, P, 3 * P).transpose(1, 0, 2)
            .reshape(P, CT * 3 * P)).astype(bf)
        in_maps.append({
            "xH": xHn,
            "wq": wqH,
            "bqkv": bslice,
            "wp": wpH,
            "bp": b_proj,
            "tri": tri,
            "ident": ident,
            "onesb": onesb,
            "onesf": onesf,
        })
    return in_maps


def kernel(x, W_qkv, b_qkv, W_proj, b_proj, _trace=False):
    from concourse import bass_utils

    if "nc" not in _CACHE:
        _CACHE["nc"] = _build_nc()
    nc = _CACHE["nc"]
    in_maps = _prep_inputs(x, W_qkv, b_qkv, W_proj, b_proj)
    res = bass_utils.run_bass_kernel_spmd(
        nc, in_maps, core_ids=list(range(NCORES)), trace=_trace,
    )
    _CACHE["last_result"] = res
    # core p rows: [b*256 + part*128 + i] = batch b, token
    # b*2048 + part*1024 + 128*p + i
    yfull = np.empty((B, T, C), dtype=np.float32)
    for p, rmap in enumerate(res.results):
        yp = rmap["y"]
        for b in range(B):
            for part in range(2):
                g0 = part * 1024 + 128 * p
                r0 = b * SL + part * P
                yfull[b, g0:g0 + P] = yp[r0:r0 + P]
    return yfull
